# revision 22
# baseline (speedup 1.0000x reference)
"""Fused quantized BasicBlock (1-bit weights / 4-bit acts) for TRN2, 8-core data-parallel.

Math: both convs see integer activations k in {0..15} (exactly representable in
fp8e4) and sign weights in {-1,0,+1}; the 3x3 conv is 9 shifted DoubleRow fp8
matmuls (K=256 contraction in one pass) accumulating exactly in fp32 PSUM.
All scalings (LSQ alpha, IR-Net weight scale, BN affine) fold into a
per-output-channel affine applied in the epilogue.

Host prep: the layer-1 input quantization min(rne(relu(x/alpha1)), 15) is a
pure element-wise integer map, so it is applied on the host (numpy, bit-exact
with the fp32 device path: same fp32 multiply and round-to-nearest-even) and
x ships as fp8 integers already in the zero-padded [58 x 57] SBUF image layout
(data at rows/cols 1..56, row stride 57, right pad of row i aliasing the left
pad of row i+1).  This cuts input DMA bytes 4x and removes the entire
on-device input-quant pipeline; a 3x3 tap (kh,kw) is a contiguous slice at
offset (r0+kh)*57+kw.  The layer-1 -> layer-2 requantization stays on device.

DMA routing (the 16 DMA engines round-robin across ALL pending descriptors of
all queues, so each queue's triggers are chained to give the earliest transfer
the full share): Sync queue carries the cc0 half of each image, GpSimd queue
the cc1 half plus all output tiles, Scalar queue the four weight tiles.  No
trigger ever shares an engine with live compute.

Startup: dummy no-dependency matmuls warm the PE clock (HAM activity window);
a PE idle gap at the dummy->real handoff drops the DVFS window to half clock,
so the dummy count is sized to overlap the first chunk's arrival.  The conv
block loop is occ-major so w1's second half is not needed until ~12us after
the first matmul.  Outputs are written as fp16 (upcast on the host).
"""

import numpy as np
import ml_dtypes

import concourse.bass as bass
import concourse.bacc as bacc
import concourse.mybir as mybir
from concourse.tile import TileContext
from concourse.tile_rust import add_dep_helper
from concourse.bass_utils import run_bass_kernel_spmd

F32 = mybir.dt.float32
F16 = mybir.dt.float16
FP8 = mybir.dt.float8e4
NP_FP8 = ml_dtypes.float8_e4m3
AF = mybir.ActivationFunctionType
ALU = mybir.AluOpType
DR = mybir.MatmulPerfMode.DoubleRow

B, C, H, W = 32, 256, 56, 56
N_CORES = 8
BPC = B // N_CORES          # images per core
PW = 57                     # padded row stride: 1 shared pad col + 56 data
NPAD = 58 * PW + 1          # bytes of padded image: rows 0..57 + corner byte
KCH = 3312                  # bytes per k-chunk (>= NPAD, multiple of 16 for DoubleRow)
NMM = 8 * PW                # psum tile size; a block's moving dim is nr*PW-1
MAGIC = float(np.float32(2.0 ** 23))  # fp32 add/sub of 2^23 == round-to-nearest-even
QMAX = 15.0
N_DUMMY = 88              # PE warm-up matmuls; sized to drain just past the
                            # first input chunk (a PE idle gap at the handoff
                            # would drop the DVFS activity window to half clock)
C0ROWS = 18                 # first k1[0] chunk: padded rows 0..17 (rb0+rb1)

_module_cache = {}


def _dep(inst, prereq, reason):
    add_dep_helper(inst.ins, prereq.ins, reason=reason)


def _emit_memset_pads(nc, kt):
    """Zero the padding borders of one [128, 2, KCH] activation tile."""
    for cc in (0, 1):
        v = kt[:, cc, :]
        nc.vector.memset(v[:, 0:PW], 0.0)                    # row 0
        nc.vector.memset(v[:, 57 * PW:KCH], 0.0)             # row 57 + tail bytes
        vv = v[:, 0:58 * PW].rearrange("p (r c) -> p r c", c=PW)
        nc.vector.memset(vv[:, 1:57, 0:1], 0.0)              # left pad col


def _emit_conv(nc, i, wt, kin, psum, layer2, ep1p, ep2p, stp, k2t, o_r, coef_t):
    """One 3x3 conv layer for image i: 9 shifted DoubleRow matmuls per output
    tile.  occ-major: all 128 output channels of occ0 stream before occ1, so
    the occ1 weight DMA has ~12us of slack after the first matmul."""
    if layer2 and i == BPC - 1:
        # split the very last row block so the final post-matmul epilogue+DMA
        # (serial tail after the last MM) is as small as possible
        blocks = [(r0, 8) for r0 in range(0, 48, 8)] + [(48, 6), (54, 2)]
    else:
        blocks = [(r0, 8) for r0 in range(0, 56, 8)]
    for occ in (0, 1):
        for r0, nr in blocks:
            nmm = nr * PW - 1
            ps = psum.tile([128, NMM], F32, tag="ps")
            for off in range(9):
                kh, kw = divmod(off, 3)
                s = (r0 + kh) * PW + kw
                nc.tensor.matmul(
                    ps[:, 0:nmm], wt[occ][:, :, off, :], kin[i][:, :, s:s + nmm],
                    start=(off == 0), stop=(off == 8), perf_mode=DR)
            psv = ps[:, 0:nr * PW].rearrange("p (r c) -> p r c", c=PW)[:, :, 0:56]
            if not layer2:
                # k2 = min(rne(relu((A1/a2)*conv + B1/a2)), 15) -> fp8, all on
                # DVE (ACT owns the layer-2 epilogue)
                t1 = ep1p.tile([128, 8 * 56], F32, tag="ep1")
                nc.vector.tensor_scalar(
                    out=t1[:, 0:nr * 56].rearrange("p (r c) -> p r c", c=56), in0=psv,
                    scalar1=coef_t[:, occ:occ + 1], scalar2=coef_t[:, 2 + occ:3 + occ],
                    op0=ALU.mult, op1=ALU.add)
                t2 = ep2p.tile([128, 8 * 56], F32, tag="ep2")
                nc.vector.tensor_scalar(
                    out=t2[:, 0:nr * 56], in0=t1[:, 0:nr * 56], scalar1=0.0,
                    scalar2=MAGIC, op0=ALU.max, op1=ALU.add)
                dst = k2t[i][:, occ, 0:58 * PW].rearrange("p (r c) -> p r c", c=PW)[
                    :, r0 + 1:r0 + 1 + nr, 1:57]
                nc.vector.tensor_scalar(
                    out=dst,
                    in0=t2[:, 0:nr * 56].rearrange("p (r c) -> p r c", c=56),
                    scalar1=MAGIC, scalar2=QMAX,
                    op0=ALU.subtract, op1=ALU.min)
            else:
                # out = relu(A2*conv + B2) as fp16 on ACT, then DMA to DRAM
                # on the GpSimd queue (inputs keep Sync; weights keep Scalar)
                st = stp.tile([128, 8 * 56], F16, tag="st")
                nc.scalar.activation(
                    out=st[:, 0:nr * 56].rearrange("p (r c) -> p r c", c=56), in_=psv,
                    func=AF.Relu, scale=coef_t[:, 4 + occ:5 + occ],
                    bias=coef_t[:, 6 + occ:7 + occ])
                nc.sync.dma_start(
                    out=o_r[i, occ][:, r0 * 56:(r0 + nr) * 56], in_=st[:, 0:nr * 56])


def _build_module():
    # Bacc (not raw Bass): its compile() legalizes multi-sem waits (TRN2 allows
    # one wait per instruction) and inserts activation table loads.
    nc = bacc.Bacc("TRN2", debug=False)
    xq_d = nc.dram_tensor("xq", [BPC, 2, 128, KCH], FP8, kind="ExternalInput")
    w1_d = nc.dram_tensor("w1p", [2, 128, 2, 9, 128], FP8, kind="ExternalInput")
    w2_d = nc.dram_tensor("w2p", [2, 128, 2, 9, 128], FP8, kind="ExternalInput")
    cf_d = nc.dram_tensor("coef", [128, 8], F32, kind="ExternalInput")
    o_d = nc.dram_tensor("out", [BPC, C, H, W], F16, kind="ExternalOutput")

    xqr = xq_d.ap()
    o_r = o_d.ap().rearrange("b (cc p) h w -> b cc p (h w)", p=128)

    with TileContext(nc) as tc:
        with tc.tile_pool(name="weights", bufs=1) as wpool, \
             tc.tile_pool(name="acts", bufs=1) as kpool, \
             tc.tile_pool(name="ep1", bufs=4) as ep1p, \
             tc.tile_pool(name="ep2", bufs=4) as ep2p, \
             tc.tile_pool(name="st", bufs=4) as stp, \
             tc.tile_pool(name="coef", bufs=1) as cfp, \
             tc.tile_pool(name="psum", bufs=8, space="PSUM") as psum:

            # PE warm-up: garbage matmuls with no data deps (beyond two tiny
            # memsets on the gpsimd engine) keep the HAM activity window busy,
            # so the first real matmuls run at 2.4 GHz instead of cold 1.2 GHz.
            dmw = wpool.tile([128, 128], FP8, tag="dmw", name="dmw")
            dmx = wpool.tile([128, 64], FP8, tag="dmx", name="dmx")
            dps = psum.tile([128, NMM], F32, tag="ps")
            nc.gpsimd.memset(dmw[:], 0.0)
            nc.gpsimd.memset(dmx[:], 0.0)
            for _ in range(N_DUMMY):
                nc.tensor.matmul(dps[:, 0:64], dmw[:], dmx[:], start=True, stop=True)

            # coef first on the Sync queue (tiny)
            coef_t = cfp.tile([128, 8], F32, tag="coef")
            nc.sync.dma_start(out=coef_t[:], in_=cf_d.ap())

            # all four weight tiles chained on the Scalar engine's queue; the
            # engine blocks on the chain, but it has no other work until the
            # first layer-2 epilogue (~40us)
            w1t, w2t = [], []
            wprev = None
            for wd, lst, nm in ((w1_d, w1t, "w1"), (w2_d, w2t, "w2")):
                for occ in (0, 1):
                    t = wpool.tile([128, 2, 9, 128], FP8, tag=f"{nm}_{occ}",
                                   name=f"{nm}_{occ}")
                    dma = nc.scalar.dma_start(out=t[:], in_=wd.ap()[occ])
                    if wprev is not None:
                        _dep(dma, wprev, "weight queue chain")
                    wprev = dma
                    lst.append(t)

            # k1 tiles arrive pre-quantized and pre-padded: pure DMA, chained
            # per queue (cc0 on Sync, cc1 on GpSimd).  Image 0 is split so the
            # first two row blocks gate on just 18 padded rows.
            k1t = [kpool.tile([128, 2, KCH], FP8, tag=f"k1_{i}", name=f"k1_{i}")
                   for i in range(BPC)]
            k2t = [kpool.tile([128, 2, KCH], FP8, tag=f"k2_{i}", name=f"k2_{i}")
                   for i in range(BPC)]
            qprev = {}

            def xdma(i, cc, lo, hi):
                eng = nc.sync if cc == 0 else nc.gpsimd
                dma = eng.dma_start(out=k1t[i][:, cc, lo:hi],
                                    in_=xqr[i, cc][:, lo:hi])
                if qprev.get(cc) is not None:
                    _dep(dma, qprev[cc], "x queue chain")
                qprev[cc] = dma

            for cc in (0, 1):
                xdma(0, cc, 0, C0ROWS * PW)
            for cc in (0, 1):
                xdma(0, cc, C0ROWS * PW, 34 * PW)
            for cc in (0, 1):
                xdma(0, cc, 34 * PW, KCH)
            for i in range(1, BPC):
                for cc in (0, 1):
                    xdma(i, cc, 0, KCH)

            # dummy activation gated only on the coef DMA: pulls the one-time
            # ACT_TABLE_LOAD off any critical path.
            scr = cfp.tile([128, 1], F32, tag="scr")
            nc.scalar.activation(out=scr[:], in_=coef_t[:, 0:1], func=AF.Relu)

            # k2 pad borders (DVE is otherwise idle until the first epilogue)
            for i in range(BPC):
                _emit_memset_pads(nc, k2t[i])

            def l1(i):
                _emit_conv(nc, i, w1t, k1t, psum, False, ep1p, ep2p, stp,
                           k2t, o_r, coef_t)

            def l2(i):
                _emit_conv(nc, i, w2t, k2t, psum, True, ep1p, ep2p, stp,
                           None, o_r, coef_t)

            l1(0); l1(1); l2(0); l1(2); l2(1); l1(3); l2(2); l2(3)

    nc.compile()
    return nc


def get_module():
    if "nc" not in _module_cache:
        _module_cache["nc"] = _build_module()
    return _module_cache["nc"]


def _binarize(w):
    """IR-Net forward: sign(normalized w) and per-out-channel scale (fp32)."""
    w = np.asarray(w, np.float32)
    mu = w.mean(axis=(1, 2, 3), keepdims=True, dtype=np.float32)
    var = ((w - mu) ** 2).mean(axis=(1, 2, 3), keepdims=True, dtype=np.float32)
    std = np.sqrt(var)
    wn = (w - mu) / (std + np.float32(1e-5))
    sgn = np.sign(wn).astype(np.float32)
    scale = np.abs(wn).mean(axis=(1, 2, 3), dtype=np.float32)  # [O]
    return sgn, scale


def _pack_weights(sgn):
    """[O=256, C=256, 3, 3] signs -> [occ, p(Ki), h(Ko), off, m] fp8 with c = h*128+p."""
    s = sgn.reshape(256, 256, 9)
    s = s.reshape(2, 128, 2, 128, 9)            # [occ, m, h, p, off]
    s = np.transpose(s, (0, 3, 2, 4, 1))        # [occ, p, h, off, m]
    return np.ascontiguousarray(s).astype(NP_FP8)


def _prep_x(x, a1):
    """Host-side layer-1 LSQ quant + padded-layout pack, bit-exact with the
    device path it replaces: fp32 multiply by (1/a1), relu, round-to-nearest-
    even, clamp to 15, as fp8 integers in the [58 x 57](+corner) layout."""
    f32 = np.float32
    xs = np.asarray(x, f32) * f32(1.0 / a1)
    q = np.minimum(np.rint(np.maximum(xs, f32(0.0)), dtype=f32), f32(QMAX))
    q = q.reshape(B, 2, 128, H, W).astype(NP_FP8)
    buf = np.zeros((B, 2, 128, KCH), NP_FP8)
    pad = buf[:, :, :, :58 * PW].reshape(B, 2, 128, 58, PW)
    pad[:, :, :, 1:57, 1:57] = q
    return buf


def kernel(x, w1, alpha1, g1, b1, m1, v1, w2, alpha2, g2, b2, m2, v2,
           _trace=False):
    f32 = np.float32
    a1 = f32(np.asarray(alpha1).reshape(()))
    a2 = f32(np.asarray(alpha2).reshape(()))
    g1, b1, m1, v1 = (np.asarray(t, f32) for t in (g1, b1, m1, v1))
    g2, b2, m2, v2 = (np.asarray(t, f32) for t in (g2, b2, m2, v2))

    s1, sc1 = _binarize(w1)
    s2, sc2 = _binarize(w2)
    inv1 = g1 / np.sqrt(v1 + f32(1e-5))
    inv2 = g2 / np.sqrt(v2 + f32(1e-5))

    A1 = (a1 * sc1 * inv1 / a2).astype(f32)         # folds layer2 1/alpha in
    B1 = ((b1 - m1 * inv1) / a2).astype(f32)
    A2 = (a2 * sc2 * inv2).astype(f32)
    B2 = (b2 - m2 * inv2).astype(f32)

    coef = np.zeros((8, 128), f32)
    coef[0:2] = A1.reshape(2, 128)
    coef[2:4] = B1.reshape(2, 128)
    coef[4:6] = A2.reshape(2, 128)
    coef[6:8] = B2.reshape(2, 128)
    coef = np.ascontiguousarray(coef.T)   # [128, 8]: contiguous per-partition DMA

    w1p = _pack_weights(s1)
    w2p = _pack_weights(s2)
    xq = _prep_x(x, a1)

    nc = get_module()
    in_maps = [
        {"xq": np.ascontiguousarray(xq[i * BPC:(i + 1) * BPC]),
         "w1p": w1p, "w2p": w2p, "coef": coef}
        for i in range(N_CORES)
    ]
    res = run_bass_kernel_spmd(nc, in_maps, core_ids=list(range(N_CORES)),
                               trace=_trace)
    out = np.concatenate([np.asarray(r["out"], np.float32)
                          for r in res.results], axis=0)
    if _trace:
        return out, res
    return out


# revision 24
# speedup vs baseline: 1.0193x; 1.0193x over previous
"""Fused quantized BasicBlock (1-bit weights / 4-bit acts) for TRN2, 8-core data-parallel.

Math: both convs see integer activations k in {0..15} (exactly representable in
fp8e4) and sign weights in {-1,0,+1}; the 3x3 conv is 9 shifted DoubleRow fp8
matmuls (K=256 contraction in one pass) accumulating exactly in fp32 PSUM.
All scalings (LSQ alpha, IR-Net weight scale, BN affine) fold into a
per-output-channel affine applied in the epilogue.

Host prep: the layer-1 input quantization min(rne(relu(x/alpha1)), 15) is a
pure element-wise integer map, so it is applied on the host (numpy, bit-exact
with the fp32 device path: same fp32 multiply and round-to-nearest-even) and
x ships as fp8 integers already in the zero-padded [58 x 57] SBUF image layout
(data at rows/cols 1..56, row stride 57, right pad of row i aliasing the left
pad of row i+1).  This cuts input DMA bytes 4x and removes the entire
on-device input-quant pipeline; a 3x3 tap (kh,kw) is a contiguous slice at
offset (r0+kh)*57+kw.  The layer-1 -> layer-2 requantization stays on device.

DMA routing (the 16 DMA engines round-robin across ALL pending descriptors of
all queues, so each queue's triggers are chained to give the earliest transfer
the full share): Sync queue carries the cc0 half of each image, GpSimd queue
the cc1 half plus all output tiles, Scalar queue the four weight tiles.  No
trigger ever shares an engine with live compute.

Startup: dummy no-dependency matmuls warm the PE clock (HAM activity window);
a PE idle gap at the dummy->real handoff drops the DVFS window to half clock,
so the dummy count is sized to overlap the first chunk's arrival.  The conv
block loop is occ-major so w1's second half is not needed until ~12us after
the first matmul.  Outputs are written as fp16 (upcast on the host).
"""

import numpy as np
import ml_dtypes

import concourse.bass as bass
import concourse.bacc as bacc
import concourse.mybir as mybir
from concourse.tile import TileContext
from concourse.tile_rust import add_dep_helper
from concourse.bass_utils import run_bass_kernel_spmd

F32 = mybir.dt.float32
F16 = mybir.dt.float16
FP8 = mybir.dt.float8e4
NP_FP8 = ml_dtypes.float8_e4m3
AF = mybir.ActivationFunctionType
ALU = mybir.AluOpType
DR = mybir.MatmulPerfMode.DoubleRow

B, C, H, W = 32, 256, 56, 56
N_CORES = 8
BPC = B // N_CORES          # images per core
PW = 57                     # padded row stride: 1 shared pad col + 56 data
NPAD = 58 * PW + 1          # bytes of padded image: rows 0..57 + corner byte
KCH = 3312                  # bytes per k-chunk (>= NPAD, multiple of 16 for DoubleRow)
NMM = 8 * PW                # psum tile size; a block's moving dim is nr*PW-1
MAGIC = float(np.float32(2.0 ** 23))  # fp32 add/sub of 2^23 == round-to-nearest-even
QMAX = 15.0
N_DUMMY = 88              # PE warm-up matmuls; sized to drain just past the
                            # first input chunk (a PE idle gap at the handoff
                            # would drop the DVFS activity window to half clock)
C0ROWS = 18                 # first k1[0] chunk: padded rows 0..17 (rb0+rb1)

_module_cache = {}


def _dep(inst, prereq, reason):
    add_dep_helper(inst.ins, prereq.ins, reason=reason)


def _emit_memset_pads(nc, kt):
    """Zero the padding borders of one [128, 2, KCH] activation tile."""
    for cc in (0, 1):
        v = kt[:, cc, :]
        nc.vector.memset(v[:, 0:PW], 0.0)                    # row 0
        nc.vector.memset(v[:, 57 * PW:KCH], 0.0)             # row 57 + tail bytes
        vv = v[:, 0:58 * PW].rearrange("p (r c) -> p r c", c=PW)
        nc.vector.memset(vv[:, 1:57, 0:1], 0.0)              # left pad col


def _emit_conv(nc, i, wt, kin, psum, layer2, ep1p, ep2p, stp, k2t, o_r, coef_t):
    """One 3x3 conv layer for image i: 9 shifted DoubleRow matmuls per output
    tile.  occ-major: all 128 output channels of occ0 stream before occ1, so
    the occ1 weight DMA has ~12us of slack after the first matmul."""
    if layer2 and i == BPC - 1:
        # split the very last row block so the final post-matmul epilogue+DMA
        # (serial tail after the last MM) is as small as possible
        blocks = [(r0, 8) for r0 in range(0, 48, 8)] + [(48, 6), (54, 2)]
    else:
        blocks = [(r0, 8) for r0 in range(0, 56, 8)]
    for occ in (0, 1):
        for r0, nr in blocks:
            nmm = nr * PW - 1
            ps = psum.tile([128, NMM], F32, tag="ps")
            for off in range(9):
                kh, kw = divmod(off, 3)
                s = (r0 + kh) * PW + kw
                nc.tensor.matmul(
                    ps[:, 0:nmm], wt[occ][:, :, off, :], kin[i][:, :, s:s + nmm],
                    start=(off == 0), stop=(off == 8), perf_mode=DR)
            psv = ps[:, 0:nr * PW].rearrange("p (r c) -> p r c", c=PW)[:, :, 0:56]
            if not layer2:
                # k2 = min(rne(relu((A1/a2)*conv + B1/a2)), 15) -> fp8, all on
                # DVE (ACT owns the layer-2 epilogue)
                t1 = ep1p.tile([128, 8 * 56], F32, tag="ep1")
                nc.vector.tensor_scalar(
                    out=t1[:, 0:nr * 56].rearrange("p (r c) -> p r c", c=56), in0=psv,
                    scalar1=coef_t[:, occ:occ + 1], scalar2=coef_t[:, 2 + occ:3 + occ],
                    op0=ALU.mult, op1=ALU.add)
                t2 = ep2p.tile([128, 8 * 56], F32, tag="ep2")
                nc.vector.tensor_scalar(
                    out=t2[:, 0:nr * 56], in0=t1[:, 0:nr * 56], scalar1=0.0,
                    scalar2=MAGIC, op0=ALU.max, op1=ALU.add)
                dst = k2t[i][:, occ, 0:58 * PW].rearrange("p (r c) -> p r c", c=PW)[
                    :, r0 + 1:r0 + 1 + nr, 1:57]
                nc.vector.tensor_scalar(
                    out=dst,
                    in0=t2[:, 0:nr * 56].rearrange("p (r c) -> p r c", c=56),
                    scalar1=MAGIC, scalar2=QMAX,
                    op0=ALU.subtract, op1=ALU.min)
            else:
                # out = relu(A2*conv + B2) as fp16 on ACT, then DMA to DRAM
                # on the GpSimd queue (inputs keep Sync; weights keep Scalar)
                st = stp.tile([128, 8 * 56], F16, tag="st")
                nc.scalar.activation(
                    out=st[:, 0:nr * 56].rearrange("p (r c) -> p r c", c=56), in_=psv,
                    func=AF.Relu, scale=coef_t[:, 4 + occ:5 + occ],
                    bias=coef_t[:, 6 + occ:7 + occ])
                nc.sync.dma_start(
                    out=o_r[i, occ][:, r0 * 56:(r0 + nr) * 56], in_=st[:, 0:nr * 56])


def _build_module():
    # Bacc (not raw Bass): its compile() legalizes multi-sem waits (TRN2 allows
    # one wait per instruction) and inserts activation table loads.
    nc = bacc.Bacc("TRN2", debug=False)
    xq_d = nc.dram_tensor("xq", [BPC, 2, 128, KCH], FP8, kind="ExternalInput")
    w1_d = nc.dram_tensor("w1p", [2, 128, 2, 9, 128], FP8, kind="ExternalInput")
    w2_d = nc.dram_tensor("w2p", [2, 128, 2, 9, 128], FP8, kind="ExternalInput")
    cf_d = nc.dram_tensor("coef", [128, 8], F32, kind="ExternalInput")
    o_d = nc.dram_tensor("out", [BPC, C, H, W], F16, kind="ExternalOutput")

    xqr = xq_d.ap()
    o_r = o_d.ap().rearrange("b (cc p) h w -> b cc p (h w)", p=128)

    with TileContext(nc) as tc:
        with tc.tile_pool(name="weights", bufs=1) as wpool, \
             tc.tile_pool(name="acts", bufs=1) as kpool, \
             tc.tile_pool(name="ep1", bufs=4) as ep1p, \
             tc.tile_pool(name="ep2", bufs=4) as ep2p, \
             tc.tile_pool(name="st", bufs=4) as stp, \
             tc.tile_pool(name="coef", bufs=1) as cfp, \
             tc.tile_pool(name="psum", bufs=8, space="PSUM") as psum:

            # PE warm-up: garbage matmuls with no data deps (beyond two tiny
            # memsets on the gpsimd engine) keep the HAM activity window busy,
            # so the first real matmuls run at 2.4 GHz instead of cold 1.2 GHz.
            dmw = wpool.tile([128, 128], FP8, tag="dmw", name="dmw")
            dmx = wpool.tile([128, 64], FP8, tag="dmx", name="dmx")
            dps = psum.tile([128, NMM], F32, tag="ps")
            nc.gpsimd.memset(dmw[:], 0.0)
            nc.gpsimd.memset(dmx[:], 0.0)
            for _ in range(N_DUMMY):
                nc.tensor.matmul(dps[:, 0:64], dmw[:], dmx[:], start=True, stop=True)

            # coef first on the Sync queue (tiny)
            coef_t = cfp.tile([128, 8], F32, tag="coef")
            nc.sync.dma_start(out=coef_t[:], in_=cf_d.ap())

            # weight tiles ride the Scalar engine's queue.  Only w1-occ0 gates
            # the first matmul; the rest are chained behind the x chunks they
            # would otherwise steal DMA-engine bandwidth from (occ-major conv
            # order gives w1-occ1 ~12us of slack, w2 ~45us).  The scalar
            # engine blocks on the chain, but it has no other work until the
            # first layer-2 epilogue (~60us).
            w1t, w2t = [], []
            wdmas = []
            for wd, lst, nm in ((w1_d, w1t, "w1"), (w2_d, w2t, "w2")):
                for occ in (0, 1):
                    t = wpool.tile([128, 2, 9, 128], FP8, tag=f"{nm}_{occ}",
                                   name=f"{nm}_{occ}")
                    dma = nc.scalar.dma_start(out=t[:], in_=wd.ap()[occ])
                    wdmas.append(dma)
                    lst.append(t)

            # k1 tiles arrive pre-quantized and pre-padded: pure DMA, chained
            # per queue (cc0 on Sync, cc1 on GpSimd).  Image 0 is split so the
            # first two row blocks gate on just 18 padded rows.
            k1t = [kpool.tile([128, 2, KCH], FP8, tag=f"k1_{i}", name=f"k1_{i}")
                   for i in range(BPC)]
            k2t = [kpool.tile([128, 2, KCH], FP8, tag=f"k2_{i}", name=f"k2_{i}")
                   for i in range(BPC)]
            qprev = {}

            def xdma(i, cc, lo, hi):
                eng = nc.sync if cc == 0 else nc.gpsimd
                dma = eng.dma_start(out=k1t[i][:, cc, lo:hi],
                                    in_=xqr[i, cc][:, lo:hi])
                if qprev.get(cc) is not None:
                    _dep(dma, qprev[cc], "x queue chain")
                qprev[cc] = dma

            for cc in (0, 1):
                xdma(0, cc, 0, C0ROWS * PW)
            for cc in (0, 1):
                xdma(0, cc, C0ROWS * PW, 34 * PW)
            for cc in (0, 1):
                xdma(0, cc, 34 * PW, KCH)
            _dep(wdmas[1], qprev[1], "w1 occ1 after image-0 input")
            for i in range(1, BPC):
                for cc in (0, 1):
                    xdma(i, cc, 0, KCH)
            _dep(wdmas[2], qprev[1], "w2 occ0 after last input")
            _dep(wdmas[3], wdmas[2], "w2 queue chain")

            # dummy activation gated only on the coef DMA: pulls the one-time
            # ACT_TABLE_LOAD off any critical path.
            scr = cfp.tile([128, 1], F32, tag="scr")
            nc.scalar.activation(out=scr[:], in_=coef_t[:, 0:1], func=AF.Relu)

            # k2 pad borders (DVE is otherwise idle until the first epilogue)
            for i in range(BPC):
                _emit_memset_pads(nc, k2t[i])

            def l1(i):
                _emit_conv(nc, i, w1t, k1t, psum, False, ep1p, ep2p, stp,
                           k2t, o_r, coef_t)

            def l2(i):
                _emit_conv(nc, i, w2t, k2t, psum, True, ep1p, ep2p, stp,
                           None, o_r, coef_t)

            l1(0); l1(1); l2(0); l1(2); l2(1); l1(3); l2(2); l2(3)

    nc.compile()
    return nc


def get_module():
    if "nc" not in _module_cache:
        _module_cache["nc"] = _build_module()
    return _module_cache["nc"]


def _binarize(w):
    """IR-Net forward: sign(normalized w) and per-out-channel scale (fp32)."""
    w = np.asarray(w, np.float32)
    mu = w.mean(axis=(1, 2, 3), keepdims=True, dtype=np.float32)
    var = ((w - mu) ** 2).mean(axis=(1, 2, 3), keepdims=True, dtype=np.float32)
    std = np.sqrt(var)
    wn = (w - mu) / (std + np.float32(1e-5))
    sgn = np.sign(wn).astype(np.float32)
    scale = np.abs(wn).mean(axis=(1, 2, 3), dtype=np.float32)  # [O]
    return sgn, scale


def _pack_weights(sgn):
    """[O=256, C=256, 3, 3] signs -> [occ, p(Ki), h(Ko), off, m] fp8 with c = h*128+p."""
    s = sgn.reshape(256, 256, 9)
    s = s.reshape(2, 128, 2, 128, 9)            # [occ, m, h, p, off]
    s = np.transpose(s, (0, 3, 2, 4, 1))        # [occ, p, h, off, m]
    return np.ascontiguousarray(s).astype(NP_FP8)


def _prep_x(x, a1):
    """Host-side layer-1 LSQ quant + padded-layout pack, bit-exact with the
    device path it replaces: fp32 multiply by (1/a1), relu, round-to-nearest-
    even, clamp to 15, as fp8 integers in the [58 x 57](+corner) layout."""
    f32 = np.float32
    xs = np.asarray(x, f32) * f32(1.0 / a1)
    q = np.minimum(np.rint(np.maximum(xs, f32(0.0)), dtype=f32), f32(QMAX))
    q = q.reshape(B, 2, 128, H, W).astype(NP_FP8)
    buf = np.zeros((B, 2, 128, KCH), NP_FP8)
    pad = buf[:, :, :, :58 * PW].reshape(B, 2, 128, 58, PW)
    pad[:, :, :, 1:57, 1:57] = q
    return buf


def kernel(x, w1, alpha1, g1, b1, m1, v1, w2, alpha2, g2, b2, m2, v2,
           _trace=False):
    f32 = np.float32
    a1 = f32(np.asarray(alpha1).reshape(()))
    a2 = f32(np.asarray(alpha2).reshape(()))
    g1, b1, m1, v1 = (np.asarray(t, f32) for t in (g1, b1, m1, v1))
    g2, b2, m2, v2 = (np.asarray(t, f32) for t in (g2, b2, m2, v2))

    s1, sc1 = _binarize(w1)
    s2, sc2 = _binarize(w2)
    inv1 = g1 / np.sqrt(v1 + f32(1e-5))
    inv2 = g2 / np.sqrt(v2 + f32(1e-5))

    A1 = (a1 * sc1 * inv1 / a2).astype(f32)         # folds layer2 1/alpha in
    B1 = ((b1 - m1 * inv1) / a2).astype(f32)
    A2 = (a2 * sc2 * inv2).astype(f32)
    B2 = (b2 - m2 * inv2).astype(f32)

    coef = np.zeros((8, 128), f32)
    coef[0:2] = A1.reshape(2, 128)
    coef[2:4] = B1.reshape(2, 128)
    coef[4:6] = A2.reshape(2, 128)
    coef[6:8] = B2.reshape(2, 128)
    coef = np.ascontiguousarray(coef.T)   # [128, 8]: contiguous per-partition DMA

    w1p = _pack_weights(s1)
    w2p = _pack_weights(s2)
    xq = _prep_x(x, a1)

    nc = get_module()
    in_maps = [
        {"xq": np.ascontiguousarray(xq[i * BPC:(i + 1) * BPC]),
         "w1p": w1p, "w2p": w2p, "coef": coef}
        for i in range(N_CORES)
    ]
    res = run_bass_kernel_spmd(nc, in_maps, core_ids=list(range(N_CORES)),
                               trace=_trace)
    out = np.concatenate([np.asarray(r["out"], np.float32)
                          for r in res.results], axis=0)
    if _trace:
        return out, res
    return out
